# revision 1
# baseline (speedup 1.0000x reference)
"""Trainium2 Bass kernel for a 2-layer GAT (graph attention network).

Strategy (8 NeuronCores, SPMD single program):
  - Nodes are partitioned contiguously across the 8 cores by destination;
    within each core the owned nodes are sorted by in-degree (descending)
    and laid out in chunks of 128 (padded-CSR layout).  All cores share one
    static per-chunk slot schedule, so the traced program is SPMD-uniform.
  - Every core builds the full layer-1 node table T1[pos] = [h | a_src |
    a_dst] (bf16, 272B rows) with one matmul per 128 nodes (the
    attention dot products are folded into the weight matrix as extra
    output columns since they are linear in x).  x arrives host-transposed.
  - Edge aggregation per chunk of 128 owned dst nodes: indirect DMAs fetch
    the source rows ([128, D, row] node-major), one 128-row gather per slot
    column (this runtime honors exactly one index per partition per
    indirect DMA).  Padding slots point at a pad row whose a_src is -1e30,
    which makes exp() underflow to exactly 0 (no contribution).
  - Softmax over slots runs on DVE/ACT per head; the gathered rows are
    scaled in place by the unnormalized attention and summed on the tensor
    engine (identity-weight accumulating matmuls into PSUM), then
    normalized by the softmax denominator.
  - The layer-2 table slice (owned nodes) is built from relu(out1) via a PE
    transpose + matmul, all-gathered across the 8 cores (bf16), and layer 2
    repeats the same gather/softmax/weighted-sum with 1 head.
  - Host does only integer graph partitioning (permutation, padded-CSR
    index arrays) and the final inverse permutation.
"""

import math

import numpy as np

# ---- problem constants (test code may override these before calling kernel) ----
N = 50000
E = 1600000
IN_CH = 128
HEADS = 4
MID = 32
OUT_CH = 64
NEG_SLOPE = 0.2
N_CORES = 8
P = 128

FT = 136                  # layer-1 table row (bf16): [h(128)|a_src(4)|a_dst(4)]
FT2 = 66                  # layer-2 table row (bf16): [h2(64)|a_src2|a_dst2]

_cache = {}
_REPS = 1                 # debug: repeat phases B..C (idempotent) to amplify
                          # device time above the measurement noise floor


def _host_prep(x, edge_index):
    n_own = N // N_CORES
    assert N % N_CORES == 0 and N_CORES % 2 == 0
    K = math.ceil(n_own / P)
    ppc = K * P
    n_pad = ppc - n_own
    n_pos = ppc * N_CORES
    assert n_pad >= 1, "need at least one pad row per core"

    src = np.asarray(edge_index[0], dtype=np.int64)
    dst = np.asarray(edge_index[1], dtype=np.int64)
    loops = np.arange(N, dtype=np.int64)
    src = np.concatenate([src, loops])
    dst = np.concatenate([dst, loops])

    deg = np.bincount(dst, minlength=N)
    core_of = np.arange(N) // n_own

    order = np.lexsort((-deg, core_of))
    pos_of_node = np.empty(N, np.int64)
    node_at_pos = np.full(n_pos, -1, np.int64)
    for c in range(N_CORES):
        nodes = order[c * n_own:(c + 1) * n_own]
        p0 = c * ppc
        pos_of_node[nodes] = p0 + np.arange(n_own)
        node_at_pos[p0:p0 + n_own] = nodes

    # CSR over dst
    eorder = np.argsort(dst, kind="stable")
    srcs_sorted = src[eorder]
    dst_sorted = dst[eorder]
    indptr = np.zeros(N + 1, np.int64)
    indptr[1:] = np.cumsum(deg)

    deg_pos = np.zeros(n_pos, np.int64)
    m = node_at_pos >= 0
    deg_pos[m] = deg[node_at_pos[m]]
    D_list = np.maximum(
        deg_pos.reshape(N_CORES, K, P).max(axis=(0, 2)), 1).astype(np.int64)
    offs = np.zeros(K + 1, np.int64)
    offs[1:] = np.cumsum(D_list)
    S = int(offs[-1])

    ranks = np.arange(len(dst_sorted)) - indptr[dst_sorted]
    pos_d = pos_of_node[dst_sorted]
    pos_s = pos_of_node[srcs_sorted]
    c_arr = pos_d // ppc
    rem = pos_d % ppc
    k_arr = rem // P
    p_arr = rem % P
    col = offs[k_arr] + ranks

    idx = np.empty((N_CORES, P, S), np.int32)
    for c in range(N_CORES):
        idx[c].fill(c * ppc + n_own)      # own first pad row (a_src=-1e30)
    idx[c_arr, p_arr, col] = pos_s

    ownpos = np.empty((N_CORES, P, K), np.int32)
    for c in range(N_CORES):
        ownpos[c] = (c * ppc + np.arange(K)[None, :] * P
                     + np.arange(P)[:, None])

    xT = np.zeros((IN_CH, n_pos), np.float32)
    xT[:, m] = np.asarray(x, np.float32)[node_at_pos[m]].T

    return dict(n_own=n_own, K=K, ppc=ppc, n_pad=n_pad, n_pos=n_pos,
                D_list=tuple(int(v) for v in D_list),
                offs=tuple(int(v) for v in offs), S=S,
                idx=idx, ownpos=ownpos, xT=xT, node_at_pos=node_at_pos)


def _aug_weights(W1, a_src1, a_dst1, W2, a_src2, a_dst2):
    W1 = np.asarray(W1, np.float32)
    W2 = np.asarray(W2, np.float32)
    A1s = np.einsum("chm,hm->ch", W1.reshape(IN_CH, HEADS, MID),
                    np.asarray(a_src1, np.float32))
    A1d = np.einsum("chm,hm->ch", W1.reshape(IN_CH, HEADS, MID),
                    np.asarray(a_dst1, np.float32))
    W1a = np.concatenate([W1, A1s, A1d], axis=1)          # [128, 136]
    A2s = W2 @ np.asarray(a_src2, np.float32).reshape(OUT_CH, 1)
    A2d = W2 @ np.asarray(a_dst2, np.float32).reshape(OUT_CH, 1)
    W2a = np.concatenate([W2, A2s, A2d], axis=1)          # [128, 66]
    return W1a, W2a


def _build_program(K, D_list, offs, S, n_pos, ppc, n_own, n_pad):
    import concourse.bass as bass
    import concourse.mybir as mybir
    import concourse.tile as tile
    from concourse.masks import make_identity

    f32 = mybir.dt.float32
    bf16 = mybir.dt.bfloat16
    i32 = mybir.dt.int32
    FTA = IN_CH + 2 * HEADS          # 136 real columns of a T1 row
    FT2A = OUT_CH + 2                # 66 real columns of a T2 row
    Alu = mybir.AluOpType
    Act = mybir.ActivationFunctionType
    IOA = bass.IndirectOffsetOnAxis

    C_W1A = 0
    C_W2A = C_W1A + FTA
    C_B1 = C_W2A + FT2A
    C_B2 = C_B1 + IN_CH
    C_PAD1 = C_B2 + OUT_CH           # [P, 8]: 0 on real rows, -1e30 on pads
    C_PAD2 = C_PAD1 + 2 * HEADS      # [P, 1]
    C_END = C_PAD2 + 1

    nc = bass.Bass(num_swdge_queues=2)
    xT = nc.declare_dram_parameter("xT", [IN_CH, n_pos], f32, isOutput=False)
    constP = nc.declare_dram_parameter("consts", [P, C_END], f32,
                                       isOutput=False)
    idxP = nc.declare_dram_parameter("idx32", [P, S], i32, isOutput=False)
    ownP = nc.declare_dram_parameter("ownpos", [P, K], i32, isOutput=False)
    outP = nc.declare_dram_parameter("out", [ppc, OUT_CH], f32, isOutput=True)

    T1 = nc.dram_tensor("T1", [n_pos, FT], bf16)
    T2s = nc.dram_tensor("T2s", [ppc, FT2], bf16)
    T2 = nc.dram_tensor("T2", [n_pos, FT2], bf16, addr_space="Shared")

    n_tiles = n_pos // P
    tiles_per_core = ppc // P        # == K

    with tile.TileContext(nc) as tc:
        with tc.tile_pool(name="const", bufs=1) as cpool:
            consts = cpool.tile([P, C_END], f32)
            nc.sync.dma_start(out=consts[:], in_=constP[:, :])
            w1a_t = consts[:, C_W1A:C_W1A + FTA]
            w2a_t = consts[:, C_W2A:C_W2A + FT2A]
            b1r_t = consts[:, C_B1:C_B1 + IN_CH]
            b2r_t = consts[:, C_B2:C_B2 + OUT_CH]
            pad1_t = consts[:, C_PAD1:C_PAD1 + 2 * HEADS]
            pad2_t = consts[:, C_PAD2:C_PAD2 + 1]
            idx_t = cpool.tile([P, S], i32)
            nc.sync.dma_start(out=idx_t[:], in_=idxP[:, :])
            own_t = cpool.tile([P, K], i32)
            nc.sync.dma_start(out=own_t[:], in_=ownP[:, :])

            ident_b = cpool.tile([P, P], bf16)
            make_identity(nc, ident_b[:])
            t2stage = cpool.tile([P, K * FT2], bf16)

            # ---------------- phase A: build T1 for all positions ----------
            NSPLIT = max(1, min(8, n_tiles // 8))
            bounds = [n_tiles * i // NSPLIT for i in range(NSPLIT + 1)]
            pp = P - (n_own % P) if n_own % P else 0      # == n_pad
            with tc.tile_pool(name="pa_x", bufs=3) as xpool, \
                 tc.tile_pool(name="pa_st", bufs=2) as stpool, \
                 tc.tile_pool(name="pa_ps", bufs=2, space="PSUM") as pspool:
                GRP = 8
                for s_ in range(NSPLIT):
                    lo, hi = bounds[s_], bounds[s_ + 1]
                    stg = stpool.tile([P, (hi - lo) * FT], bf16, tag="stg")
                    for t0 in range(lo, hi, GRP):
                        g = min(GRP, hi - t0)
                        xt = xpool.tile([IN_CH, g * P], f32, tag="xt")
                        nc.sync.dma_start(out=xt[:],
                                          in_=xT[:, t0 * P:(t0 + g) * P])
                        for j in range(g):
                            ps = pspool.tile([P, FTA], f32, tag="ps")
                            nc.tensor.matmul(ps[:],
                                             lhsT=xt[:, j * P:(j + 1) * P],
                                             rhs=w1a_t, start=True, stop=True)
                            t = t0 + j
                            nc.scalar.copy(
                                stg[:, (t - lo) * FT:(t - lo) * FT + FTA],
                                ps[:])
                            # pad rows of any core slice inside this tile:
                            # a_src/a_dst += -1e30 (additive mask, 0 on
                            # real rows; pad rows have h == a == 0)
                            if (t % tiles_per_core) == tiles_per_core - 1 \
                                    and n_pad:
                                nc.vector.tensor_add(
                                    stg[:, (t - lo) * FT + IN_CH:
                                        (t - lo) * FT + IN_CH + 2 * HEADS],
                                    ps[:, IN_CH:IN_CH + 2 * HEADS],
                                    pad1_t)
                    dview = T1[lo * P:hi * P, :].rearrange(
                        "(t p) f -> p t f", p=P)
                    nc.sync.dma_start(
                        out=dview,
                        in_=stg[:].rearrange("p (t f) -> p t f", f=FT))

            for _rep in range(_REPS):
                # ---------------- phase B: layer-1 aggregation ------------------
                with tc.tile_pool(name="pb_g", bufs=6) as gpool, \
                     tc.tile_pool(name="pb_sm", bufs=3) as smpool, \
                     tc.tile_pool(name="pb_ps", bufs=2, space="PSUM") as psB, \
                     tc.tile_pool(name="pb_pst", bufs=2, space="PSUM") as psT, \
                     tc.tile_pool(name="pb_psu", bufs=2, space="PSUM") as psU:
                    for k in range(K):
                        D = D_list[k]
                        co = offs[k]
                        G = gpool.tile([P, D * FT], bf16, tag="G")
                        G3 = G[:].rearrange("p (d f) -> p d f", f=FT)
                        for d in range(D):
                            inst = nc.gpsimd.indirect_dma_start(
                                out=G3[:, d], out_offset=None, in_=T1[:, :],
                                in_offset=IOA(ap=idx_t[:, co + d:co + d + 1],
                                              axis=0))
                            if d % 2:
                                inst.ins.queue = "qPoolDynamic1"
                        ownr = smpool.tile([P, FT], bf16, tag="ownr")
                        nc.gpsimd.indirect_dma_start(
                            out=ownr[:], out_offset=None, in_=T1[:, :],
                            in_offset=IOA(ap=own_t[:, k:k + 1], axis=0))
                        adst = smpool.tile([P, HEADS], f32, tag="adst")
                        nc.vector.tensor_copy(
                            adst[:], ownr[:, IN_CH + HEADS:IN_CH + 2 * HEADS])

                        logits = smpool.tile([P, HEADS * D], f32, tag="logits")
                        for h in range(HEADS):
                            lh = logits[:, h * D:(h + 1) * D]
                            asrc_h = G3[:, :, IN_CH + h:IN_CH + h + 1].squeeze(2)
                            nc.vector.tensor_scalar_add(lh, asrc_h, adst[:, h:h + 1])
                            nc.vector.scalar_tensor_tensor(
                                lh, lh, NEG_SLOPE, lh, op0=Alu.mult, op1=Alu.max)
                        negmax = smpool.tile([P, HEADS], f32, tag="negmax")
                        for h in range(HEADS):
                            nc.vector.reduce_max(
                                negmax[:, h:h + 1], logits[:, h * D:(h + 1) * D],
                                axis=mybir.AxisListType.X, negate=True)
                        e_t = smpool.tile([P, HEADS * D], f32, tag="e")
                        s_t = smpool.tile([P, HEADS], f32, tag="s")
                        for h in range(HEADS):
                            nc.scalar.activation(
                                e_t[:, h * D:(h + 1) * D],
                                logits[:, h * D:(h + 1) * D],
                                Act.Exp, bias=negmax[:, h:h + 1],
                                accum_out=s_t[:, h:h + 1])
                        rcp = smpool.tile([P, HEADS], f32, tag="rcp")
                        nc.vector.reciprocal(rcp[:], s_t[:])

                        # scale gathered h in place by unnormalized attention
                        hview = G3[:, :, 0:IN_CH].rearrange(
                            "p d (h c) -> p d h c", c=MID)
                        e_b = e_t[:].rearrange("p (h d) -> p d h", d=D) \
                            .unsqueeze(3).to_broadcast([P, D, HEADS, MID])
                        nc.vector.tensor_tensor(out=hview, in0=hview, in1=e_b,
                                                op=Alu.mult)

                        ps = psB.tile([P, IN_CH], f32, tag="acc")
                        for d in range(D):
                            nc.tensor.matmul(ps[:], lhsT=ident_b[:],
                                             rhs=G3[:, d, 0:IN_CH],
                                             start=(d == 0), stop=(d == D - 1))

                        tmp = smpool.tile([P, IN_CH], f32, tag="tmp")
                        rcp_b = rcp[:].unsqueeze(2).to_broadcast([P, HEADS, MID])
                        nc.vector.tensor_tensor(
                            out=tmp[:].rearrange("p (h c) -> p h c", c=MID),
                            in0=ps[:].rearrange("p (h c) -> p h c", c=MID),
                            in1=rcp_b, op=Alu.mult)
                        nc.vector.tensor_add(tmp[:], tmp[:], b1r_t)
                        r1 = smpool.tile([P, IN_CH], bf16, tag="r1")
                        nc.scalar.activation(r1[:], tmp[:], Act.Relu)

                        tps = psT.tile([P, P], bf16, tag="tps")
                        nc.tensor.transpose(tps[:], r1[:], ident_b[:])
                        r1T = smpool.tile([P, P], f32, tag="r1T")
                        nc.vector.tensor_copy(r1T[:], tps[:])
                        t2ps = psU.tile([P, FT2A], f32, tag="t2ps")
                        nc.tensor.matmul(t2ps[:], lhsT=r1T[:], rhs=w2a_t,
                                         start=True, stop=True)
                        nc.scalar.copy(t2stage[:, k * FT2:k * FT2 + FT2A], t2ps[:])
                        # own pad rows (last chunk): a_src2 += -1e30
                        if k == K - 1 and n_pad:
                            nc.vector.tensor_add(
                                t2stage[:, k * FT2 + OUT_CH:k * FT2 + OUT_CH + 1],
                                t2ps[:, OUT_CH:OUT_CH + 1],
                                pad2_t)

                nc.sync.dma_start(
                    out=T2s[:, :].rearrange("(k p) f -> p k f", p=P),
                    in_=t2stage[:].rearrange("p (k f) -> p k f", f=FT2))

                nc.gpsimd.collective_compute(
                    "AllGather",
                    mybir.AluOpType.bypass,
                    replica_groups=[list(range(N_CORES))],
                    ins=[T2s[:, :]],
                    outs=[T2[:, :]],
                )

                # ---------------- phase C: layer-2 aggregation ------------------
                with tc.tile_pool(name="pc_g", bufs=6) as g2pool, \
                     tc.tile_pool(name="pc_sm", bufs=3) as sm2pool, \
                     tc.tile_pool(name="pc_ps", bufs=2, space="PSUM") as psC:
                    for k in range(K):
                        D = D_list[k]
                        co = offs[k]
                        G2 = g2pool.tile([P, D * FT2], bf16, tag="G2")
                        G23 = G2[:].rearrange("p (d f) -> p d f", f=FT2)
                        for d in range(D):
                            inst = nc.gpsimd.indirect_dma_start(
                                out=G23[:, d], out_offset=None, in_=T2[:, :],
                                in_offset=IOA(ap=idx_t[:, co + d:co + d + 1],
                                              axis=0))
                            if d % 2:
                                inst.ins.queue = "qPoolDynamic1"
                        adst2 = sm2pool.tile([P, 1], f32, tag="adst2")
                        nc.vector.tensor_copy(
                            adst2[:],
                            t2stage[:, k * FT2 + OUT_CH + 1:k * FT2 + OUT_CH + 2])

                        logits2 = sm2pool.tile([P, D], f32, tag="logits2")
                        asrc2 = G23[:, :, OUT_CH:OUT_CH + 1].squeeze(2)
                        nc.vector.tensor_scalar_add(logits2[:], asrc2, adst2[:, 0:1])
                        nc.vector.scalar_tensor_tensor(
                            logits2[:], logits2[:], NEG_SLOPE, logits2[:],
                            op0=Alu.mult, op1=Alu.max)
                        negmax2 = sm2pool.tile([P, 1], f32, tag="negmax2")
                        nc.vector.reduce_max(negmax2[:], logits2[:],
                                             axis=mybir.AxisListType.X, negate=True)
                        e2 = sm2pool.tile([P, D], f32, tag="e2")
                        s2 = sm2pool.tile([P, 1], f32, tag="s2")
                        nc.scalar.activation(e2[:], logits2[:], Act.Exp,
                                             bias=negmax2[:, 0:1],
                                             accum_out=s2[:, 0:1])
                        rcp2 = sm2pool.tile([P, 1], f32, tag="rcp2")
                        nc.vector.reciprocal(rcp2[:], s2[:])

                        h2view = G23[:, :, 0:OUT_CH]
                        e2_b = e2[:].unsqueeze(2).to_broadcast([P, D, OUT_CH])
                        nc.vector.tensor_tensor(out=h2view, in0=h2view, in1=e2_b,
                                                op=Alu.mult)

                        ps2 = psC.tile([P, OUT_CH], f32, tag="acc2")
                        for d in range(D):
                            nc.tensor.matmul(ps2[:], lhsT=ident_b[:],
                                             rhs=G23[:, d, 0:OUT_CH],
                                             start=(d == 0), stop=(d == D - 1))

                        outt = sm2pool.tile([P, OUT_CH], f32, tag="outt")
                        nc.scalar.activation(outt[:], ps2[:], Act.Identity,
                                             scale=rcp2[:, 0:1])
                        nc.vector.tensor_add(outt[:], outt[:], b2r_t)
                        nc.sync.dma_start(out=outP[k * P:(k + 1) * P, :],
                                          in_=outt[:])

    _split_excess_waits(nc, mybir)
    return nc


def _split_excess_waits(nc, mybir):
    """Walrus allows only one sync-wait command per instruction here.
    Hoist excess waits onto freshly inserted same-engine NoOps (safe:
    waiting earlier on the same engine)."""
    ctr = 0
    for bb in nc.main_func.blocks:
        out = []
        changed = False
        for ins in bb.instructions:
            si = ins.sync_info
            waits = list(si.on_wait) if (si is not None and si.on_wait) else []
            if len(waits) > 1:
                keep = waits[-1:]
                excess = waits[:-1]
                for w in excess:
                    ctr += 1
                    nop = mybir.InstNoOp(
                        name=f"waitsplit-{ctr}-{ins.name}",
                        opcode="NoOp",
                        engine=ins.engine,
                        sync_info=mybir.SyncInfo(on_wait=[w], on_update=[]),
                    )
                    out.append(nop)
                ins.sync_info = mybir.SyncInfo(
                    on_wait=keep,
                    on_update=list(si.on_update) if si.on_update else [])
                changed = True
            out.append(ins)
        if changed:
            try:
                bb.instructions[:] = out
            except TypeError:
                bb.instructions = out


def _make_runner(nc, n_cores):
    import jax
    from jax.sharding import Mesh, PartitionSpec
    from jax.experimental.shard_map import shard_map
    from concourse import bass2jax
    import concourse.mybir as mybir

    bass2jax.install_neuronx_cc_hook()
    partition_name = (nc.partition_id_tensor.name
                      if nc.partition_id_tensor else None)
    in_names = []
    out_names = []
    out_avals = []
    zero_outs = []
    for alloc in nc.m.functions[0].allocations:
        if not isinstance(alloc, mybir.MemoryLocationSet):
            continue
        name = alloc.memorylocations[0].name
        if alloc.kind == "ExternalInput":
            if name != partition_name:
                in_names.append(name)
        elif alloc.kind == "ExternalOutput":
            shape = tuple(alloc.tensor_shape)
            dtype = mybir.dt.np(alloc.dtype)
            out_names.append(name)
            out_avals.append(jax.core.ShapedArray(shape, dtype))
            zero_outs.append(np.zeros(shape, dtype))
    n_params = len(in_names)
    all_names = list(in_names) + out_names
    if partition_name is not None:
        all_names.append(partition_name)

    def _body(*args):
        operands = list(args)
        if partition_name is not None:
            operands.append(bass2jax.partition_id_tensor())
        outs = bass2jax._bass_exec_p.bind(
            *operands,
            out_avals=tuple(out_avals),
            in_names=tuple(all_names),
            out_names=tuple(out_names),
            lowering_input_output_aliases=(),
            sim_require_finite=True,
            sim_require_nnan=True,
            nc=nc,
        )
        return tuple(outs)

    devices = jax.devices()[:n_cores]
    mesh = Mesh(np.asarray(devices), ("core",))
    nio = n_params + len(out_names)
    sharded = jax.jit(
        shard_map(_body, mesh=mesh, in_specs=(PartitionSpec("core"),) * nio,
                  out_specs=(PartitionSpec("core"),) * len(out_names),
                  check_rep=False),
        keep_unused=True,
    )
    return dict(fn=sharded, in_names=in_names, out_names=out_names,
                zero_outs=zero_outs, mesh=mesh, n_cores=n_cores)


def _execute(runner, in_maps):
    import jax
    n_cores = runner["n_cores"]
    concat_in = [
        np.concatenate([np.asarray(in_maps[c][name])
                        for c in range(n_cores)], axis=0)
        for name in runner["in_names"]
    ]
    concat_zeros = [
        np.zeros((n_cores * z.shape[0], *z.shape[1:]), z.dtype)
        for z in runner["zero_outs"]
    ]
    out_arrs = runner["fn"](*concat_in, *concat_zeros)
    out_arrs = [np.asarray(a) for a in out_arrs]
    res = []
    for c in range(n_cores):
        m = {}
        for i, name in enumerate(runner["out_names"]):
            a = out_arrs[i]
            s0 = a.shape[0] // n_cores
            m[name] = a[c * s0:(c + 1) * s0]
        res.append(m)
    return res


def _time_exec(runner, in_maps, iters=5):
    """Steady-state wall-clock of the compiled NEFF execution (device-resident
    inputs, no host transfers in the loop)."""
    import time as _time

    import jax
    from jax.sharding import NamedSharding, PartitionSpec

    n_cores = runner["n_cores"]
    sh = NamedSharding(runner["mesh"], PartitionSpec("core"))
    concat_in = [
        np.concatenate([np.asarray(in_maps[c][name])
                        for c in range(n_cores)], axis=0)
        for name in runner["in_names"]
    ]
    concat_zeros = [
        np.zeros((n_cores * z.shape[0], *z.shape[1:]), z.dtype)
        for z in runner["zero_outs"]
    ]
    dev_in = [jax.device_put(a, sh) for a in concat_in]
    dev_z = [jax.device_put(a, sh) for a in concat_zeros]
    times = []
    for _ in range(iters):
        t0 = _time.perf_counter()
        outs = runner["fn"](*dev_in, *dev_z)
        for o in outs:
            o.block_until_ready()
        times.append(_time.perf_counter() - t0)
    return min(times), times


def _get_compiled(inputs):
    x = np.asarray(inputs["x"], np.float32)
    prep = _host_prep(x, np.asarray(inputs["edge_index"]))
    key = (prep["K"], prep["D_list"], prep["n_pos"], prep["ppc"],
           prep["n_own"], prep["n_pad"], _REPS)
    if key not in _cache:
        nc = _build_program(prep["K"], prep["D_list"], prep["offs"],
                            prep["S"], prep["n_pos"], prep["ppc"],
                            prep["n_own"], prep["n_pad"])
        _cache[key] = _make_runner(nc, N_CORES)
    runner = _cache[key]

    W1a, W2a = _aug_weights(inputs["W1"], inputs["a_src1"], inputs["a_dst1"],
                            inputs["W2"], inputs["a_src2"], inputs["a_dst2"])
    b1r = np.broadcast_to(np.asarray(inputs["b1"], np.float32),
                          (P, IN_CH)).copy()
    b2r = np.broadcast_to(np.asarray(inputs["b2"], np.float32),
                          (P, OUT_CH)).copy()
    FTA = IN_CH + 2 * HEADS
    FT2A = OUT_CH + 2
    consts = np.zeros((P, FTA + FT2A + IN_CH + OUT_CH + 2 * HEADS + 1),
                      np.float32)
    o = 0
    consts[:IN_CH, o:o + FTA] = W1a
    o += FTA
    consts[:IN_CH, o:o + FT2A] = W2a
    o += FT2A
    consts[:, o:o + IN_CH] = b1r
    o += IN_CH
    consts[:, o:o + OUT_CH] = b2r
    o += OUT_CH
    # additive pad masks: -1e30 on pad partitions of each slice's last tile
    padrow = np.zeros(P, np.float32)
    r = prep["n_own"] % P
    if prep["n_pad"]:
        padrow[r:] = -1e30
    consts[:, o:o + 2 * HEADS] = padrow[:, None]
    o += 2 * HEADS
    consts[:, o:o + 1] = padrow[:, None]

    in_maps = []
    for c in range(N_CORES):
        in_maps.append({
            "xT": prep["xT"],
            "consts": consts,
            "idx32": prep["idx"][c],
            "ownpos": prep["ownpos"][c],
        })
    return runner, in_maps, prep


def _run(inputs):
    runner, in_maps, prep = _get_compiled(inputs)
    # transient NRT_EXEC_UNIT_UNRECOVERABLE hiccups have been observed on
    # this runtime; back off and retry a couple of times
    import time as _time
    last_exc = None
    for attempt in range(3):
        try:
            results = _execute(runner, in_maps)
            break
        except Exception as exc:
            last_exc = exc
            _time.sleep(2.0 + 4.0 * attempt)
    else:
        raise last_exc
    out = np.empty((N, OUT_CH), np.float32)
    n_own, ppc = prep["n_own"], prep["ppc"]
    for c in range(N_CORES):
        o = np.asarray(results[c]["out"])
        nodes = prep["node_at_pos"][c * ppc:c * ppc + n_own]
        out[nodes] = o[:n_own]
    return out


def kernel(**inputs):
    return _run(inputs)

